# revision 1
# baseline (speedup 1.0000x reference)
"""Trainium2 Bass kernel for nn_Attention3D (GroupNorm -> QKV -> MHA -> proj -> residual).

Sharding: one (batch, head) pair per NeuronCore (B=2 x HEADS=4 = 8 cores).
Each core computes GroupNorm(x[b]) (recomputed per core, cheap), its head's
Q/K/V, the full 4096x4096 attention for its (b,h), the head's slice of the
output projection, plus a quarter of the residual+bias. The host sums the 4
per-head partials per batch.

On-chip layout: S^T = K^T Q is computed with the softmax (key) dim on PSUM
partitions so the softmax denominator comes out of the AV matmul itself via a
ones-column appended to V^T (no transposes anywhere). exp() runs on the
scalar engine in fp32; all matmul operands are bf16 with fp32 PSUM accumulation.

Raw Bass (no Tile): this toolchain's walrus build only supports one embedded
sem-wait and one sem-update per instruction, so scheduling uses one monotone
counting semaphore per engine with a two-pass (count, then emit) scheduler.
"""

import numpy as np
import ml_dtypes

import concourse.bass as bass
import concourse.mybir as mybir
from concourse.bass_utils import run_bass_kernel_spmd

F32 = mybir.dt.float32
BF16 = mybir.dt.bfloat16
AF = mybir.ActivationFunctionType
OP = mybir.AluOpType

# problem constants (hardcoded per contract)
B, C, D, H, W = 2, 256, 16, 16, 16
N = D * H * W            # 4096
HEADS = 4
HD = C // HEADS          # 64
GROUPS = 8
EPS = 1e-5
SCALE = HD ** -0.5

NCH = 8                  # n-chunks of 512 (query dim)
CHW = 512
NMB = 32                 # m-blocks of 128 (key dim)
MBW = 128
NE = NCH * NMB           # 256 inner iterations
LAG = 4                  # mm_av trails mm_s by LAG iterations
TAIL_DELAY = 8           # proj tail trails mm_av by this many iterations
VLOOK = 4                # V^T blocks computed in preamble; rest in-loop
SROT = 3                 # psum S^T bank rotation
PROT = 4                 # P^T sbuf tile rotation

# vb (f32 vector blob) column layout
VB_BQK = 0       # rows 0:64 = bq*scale, rows 64:128 = bk
VB_BV = 1
VB_BP = 2        # 2,3 : proj bias * 0.25 per c-tile
VB_GNW = 4       # 4,5
VB_GNB = 6       # 6,7
VB_GM = 8        # 8:16 t0, 16:24 t1  group mask [128,8]
VB_BM = 24       # 24:152 t0, 152:280 t1  bcast mask rows 0:8 [8,128]
VB_ONES = 280    # 280:344 row 0 ones [1,64]
VB_EPS = 344
VB_COLS = 352

# wb (bf16 weight blob) column layout
WB_WQK = 0       # [128,128] per c-tile: cols 0:64 = (Wq*scale).T, 64:128 = Wk.T
WB_WV = 256      # 256:320 t0, 320:384 t1
WB_WP = 384      # 384:640  WpT [64,256] on rows 0:64
WB_COLS = 640


def _wr_update(inst, sem, val):
    u = mybir.SyncUpdate(sync_type='semaphore', id=sem.num, ant_name=None,
                         update_mode='sem-wr-imm', update_value=val)
    si = inst.ins.sync_info
    if si is None:
        inst.ins.sync_info = mybir.SyncInfo(on_wait=[], on_update=[u])
    else:
        si.on_update.append(u)
    return inst


def _sub_update(inst, sem, val):
    u = mybir.SyncUpdate(sync_type='semaphore', id=sem.num, ant_name=None,
                         update_mode='sem-sub-imm', update_value=val)
    si = inst.ins.sync_info
    if si is None:
        inst.ins.sync_info = mybir.SyncInfo(on_wait=[], on_update=[u])
    else:
        si.on_update.append(u)
    return inst


class Sched:
    """Two-pass static scheduler: pass 0 counts per-engine sem positions and
    records named events; pass 1 emits instructions with embedded waits."""

    def __init__(self):
        self.ev = {}
        self.emitting = False
        self.cnt = {}
        self.sem = {}

    def reset_counts(self):
        self.cnt = {k: 0 for k in list(("pe", "act", "dve", "dw", "st0", "st1")) + [f"dxc{t}{j}" for t in range(2) for j in range(4)]}

    def bump(self, key, n, ev=None):
        self.cnt[key] += n
        if not self.emitting:
            if ev is not None:
                self.ev[ev] = (key, self.cnt[key])
        return self.cnt[key]

    def attach(self, inst, key, n, ev=None, wait=None):
        """Attach the engine-counter inc (and optional embedded wait) to inst."""
        if self.emitting:
            if wait is not None:
                wsem, wval = self.ev[wait]
                inst._wait_ge(self.sem[wsem], wval)
            inst.then_inc(self.sem[key], n)
        self.bump(key, n, ev)

    def wval(self, evname):
        return self.ev[evname]


def build_module(debug=False, srot=SROT, lag=LAG, prot=PROT, self_waits=True, ablate=(), zeros=True, chunk_aff=True, chunk_dma=True, finalizer=True):
    nc = bass.Bass()

    x_d = nc.dram_tensor("x", [C, N], F32, kind="ExternalInput")
    wb_d = nc.dram_tensor("wb", [128, WB_COLS], BF16, kind="ExternalInput")
    vb_d = nc.dram_tensor("vb", [128, VB_COLS], F32, kind="ExternalInput")
    out_d = nc.dram_tensor("out", [C, N], F32, kind="ExternalOutput")
    if debug:
        dbg_xn = nc.dram_tensor("dbg_xn", [C, N], BF16, kind="ExternalOutput")
        dbg_q = nc.dram_tensor("dbg_q", [64, N], BF16, kind="ExternalOutput")
        dbg_k = nc.dram_tensor("dbg_k", [64, N], BF16, kind="ExternalOutput")
        dbg_v = nc.dram_tensor("dbg_v", [128, NMB * 66], BF16, kind="ExternalOutput")
        dbg_avn = nc.dram_tensor("dbg_avn", [64, 2 * CHW], BF16, kind="ExternalOutput")
        dbg_r = nc.dram_tensor("dbg_r", [64, 2 * CHW], F32, kind="ExternalOutput")
        dbg_gn = nc.dram_tensor("dbg_gn", [128, 16], F32, kind="ExternalOutput")
        dbg_bn = nc.dram_tensor("dbg_bn", [128, 50], F32, kind="ExternalOutput")

    from contextlib import ExitStack
    es = ExitStack()

    # ---- PSUM: preamble phase (freed before the attention loop allocs) ----
    with ExitStack() as pre:
        pqk = pre.enter_context(nc.psum_tensor("pqk", [128, CHW], F32)).ap()
        pv0 = pre.enter_context(nc.psum_tensor("pv0", [128, CHW], F32)).ap()
        pv1 = pre.enter_context(nc.psum_tensor("pv1", [128, CHW], F32)).ap()
        paux = pre.enter_context(nc.psum_tensor("paux", [128, CHW], F32)).ap()
    pv = [pv0, pv1]  # rebound to ps_p corners below once loop psum exists
    gs_ps = paux[0:8, 0:2]            # group (sum mean, sum m2+mu^2)
    cb_ps = [paux[:, 2:4], paux[:, 4:6]]  # per-channel (mu, rstd) per c-tile

    # ---- PSUM: attention loop phase ----
    # S^T tiles live in two double-bank tensors so exp can process two
    # m-blocks per ACTIVATE (amortizes the ~300ns fixed cost per op).
    with ExitStack() as lp:
        ps_s2 = [lp.enter_context(nc.psum_tensor(f"ps{i}", [128, 2 * CHW], F32)).ap()
                 for i in range(2)]
        ps_o = [lp.enter_context(nc.psum_tensor(f"po{i}", [128, CHW], F32)).ap()
                for i in range(2)]
        ps_p = [lp.enter_context(nc.psum_tensor(f"pp{i}", [128, CHW], F32)).ap()
                for i in range(2)]
    # V^T staging alternates between two DIFFERENT banks (same-bank PE-write
    # + DVE-read is a fatal hardware hazard): ps_o[1] is idle through chunk 0
    # (first AV use at e=36) and ps_p[0] until proj(0) (e~44); all in-loop
    # V^T blocks land by e~30.
    pv = [ps_o[1][:, 448:512], ps_p[0][:, 448:512]]

    # ---- SBUF ----
    x_sb = [es.enter_context(nc.sbuf_tensor(f"x{t}", [128, N], F32)).ap()
            for t in range(2)]
    xn_sb = [es.enter_context(nc.sbuf_tensor(f"xn{t}", [128, N], BF16)).ap()
             for t in range(2)]
    q_sb = es.enter_context(nc.sbuf_tensor("q", [128, N], BF16)).ap()
    k_sb = es.enter_context(nc.sbuf_tensor("k", [128, N], BF16)).ap()
    vaug = es.enter_context(nc.sbuf_tensor("vaug", [128, NMB, 66], BF16)).ap()
    pt_sb = es.enter_context(nc.sbuf_tensor("pt", [128, 4, 2 * CHW], BF16)).ap()
    avn_sb = es.enter_context(nc.sbuf_tensor("avn", [64, 2, CHW], BF16)).ap()
    ost_sb = es.enter_context(nc.sbuf_tensor("ost", [128, 2, 2, CHW], F32)).ap()
    wb_sb = es.enter_context(nc.sbuf_tensor("wbs", [128, WB_COLS], BF16)).ap()
    vb_sb = es.enter_context(nc.sbuf_tensor("vbs", [128, VB_COLS], F32)).ap()
    stats_sb2 = [es.enter_context(nc.sbuf_tensor(f"stats{t}", [128, 8, 6], F32)).ap()
                 for t in range(2)]
    mv_sb = es.enter_context(nc.sbuf_tensor("mv", [128, 2], F32)).ap()
    st2_sb = es.enter_context(nc.sbuf_tensor("st2", [128, 2, 2], F32)).ap()
    musq_sb = es.enter_context(nc.sbuf_tensor("musq", [128, 1], F32)).ap()
    g8_sb = es.enter_context(nc.sbuf_tensor("g8", [8, 6], F32)).ap()
    # g8 cols: 0,1 = gtmp(mu, Ex2); 2 = var; 3 = sd; 4 = rstd
    gst2_sb = es.enter_context(nc.sbuf_tensor("gst2", [8, 2], F32)).ap()
    coef_sb = es.enter_context(nc.sbuf_tensor("coef", [128, 2, 2], F32)).ap()
    tmp1_sb = es.enter_context(nc.sbuf_tensor("tmp1", [128, 1], F32)).ap()
    warm_sb = es.enter_context(nc.sbuf_tensor("warm", [1, 1], F32)).ap()
    r64_sb = es.enter_context(nc.sbuf_tensor("r64", [64, 2, CHW], F32)).ap()

    sems = {}
    for name in ["pe", "act", "dve", "dw", "st0", "st1"] + [f"dxc{t}{j}" for t in range(2) for j in range(4)] + ["fin"]:
        sems[name] = es.enter_context(nc.semaphore(f"sem_{name}"))

    s = Sched()
    s.sem = sems

    wqk_w = [wb_sb[:, WB_WQK + 128 * t: WB_WQK + 128 * (t + 1)] for t in range(2)]
    wv_w = [wb_sb[:, WB_WV + 64 * t: WB_WV + 64 * (t + 1)] for t in range(2)]
    wp_w = [wb_sb[0:64, WB_WP + 128 * t: WB_WP + 128 * (t + 1)] for t in range(2)]
    gm_w = [vb_sb[:, VB_GM + 8 * t: VB_GM + 8 * (t + 1)] for t in range(2)]
    bm_w = [vb_sb[0:8, VB_BM + 128 * t: VB_BM + 128 * (t + 1)] for t in range(2)]
    ones_w = vb_sb[0:1, VB_ONES: VB_ONES + 64]
    bq_v = vb_sb[0:64, VB_BQK: VB_BQK + 1]
    bk_v = vb_sb[64:128, VB_BQK: VB_BQK + 1]
    bv_v = vb_sb[0:64, VB_BV: VB_BV + 1]
    bp_v = [vb_sb[:, VB_BP + t: VB_BP + t + 1] for t in range(2)]
    gnw_v = [vb_sb[:, VB_GNW + t: VB_GNW + t + 1] for t in range(2)]
    gnb_v = [vb_sb[:, VB_GNB + t: VB_GNB + t + 1] for t in range(2)]

    def zero_sems(eng, names):
        # Device semaphores persist across executions; each engine write-zeroes
        # the sems it owns at the head of its program (ordered before any of
        # its increments; normal repeat runs are already zeroed by the
        # end-of-run subtract finalizer, this guards abnormal prior state).
        if s.emitting and zeros:
            for name in names:
                _wr_update(eng.wait_ge(sems[name], 0), sems[name], 0)

    # ---------------- engine programs ----------------

    def gen_sync(eng):
        def dma(key, out, in_, ev=None, wait=None):
            if s.emitting:
                i = nc.sync.dma_start(out=out, in_=in_)
                s.attach(i, key, 16, ev=ev, wait=wait)
            else:
                s.bump(key, 16, ev)

        zero_sems(eng, ["dw", "st0", "st1"]
                  + [f"dxc{t}{j}" for t in range(2) for j in range(4)])
        dma("dw", wb_sb, wb_d[:, :], ev="wb")
        dma("dw", vb_sb, vb_d[:, :], ev="vb")
        if chunk_dma:
            for t in range(2):
                for j in range(4):
                    dma(f"dxc{t}{j}", x_sb[t][:, 1024 * j:1024 * (j + 1)],
                        x_d[128 * t:128 * (t + 1), 1024 * j:1024 * (j + 1)],
                        ev=f"x{t}c{j}")
        else:
            for t in range(2):
                dma(f"dxc{t}0", x_sb[t], x_d[128 * t:128 * (t + 1), :],
                    ev=f"x{t}w")
            if not s.emitting:
                for t in range(2):
                    for j in range(4):
                        s.ev[f"x{t}c{j}"] = s.ev[f"x{t}w"]
        for ch in range(NCH):
            for t in range(2):
                dma(f"st{ch % 2}",
                    out_d[128 * t:128 * (t + 1), CHW * ch: CHW * (ch + 1)],
                    ost_sb[:, ch % 2, t, :], ev=f"store{ch}_{t}",
                    wait=f"ocopy{ch}_{t}")
        if s.emitting:
            eng.wait_ge(sems["st0"], s.cnt["st0"])
            eng.wait_ge(sems["st1"], s.cnt["st1"])
            st0_extra = 0
        if debug and s.emitting:
            eng.wait_ge(sems["dve"], s.cnt["dve"])
            eng.wait_ge(sems["act"], s.cnt["act"])
            ndbg = 0
            for t in range(2):
                nc.sync.dma_start(out=dbg_xn[128 * t:128 * (t + 1), :],
                                  in_=xn_sb[t]).then_inc(sems["st0"], 16)
                ndbg += 1
            nc.sync.dma_start(out=dbg_bn[:, 0:48],
                              in_=stats_sb.rearrange("p a b -> p (a b)")).then_inc(sems["st0"], 16)
            nc.sync.dma_start(out=dbg_bn[:, 48:50], in_=mv_sb).then_inc(sems["st0"], 16)
            ndbg += 2
            gnpack = [(st2_sb.rearrange("p a b -> p (a b)"), 0, 4),
                      (g8_sb, 4, 10), (gst2_sb, 10, 12),
                      (coef_sb.rearrange("p a b -> p (a b)"), 12, 16)]
            for srcap, c0, c1 in gnpack:
                nc.sync.dma_start(out=dbg_gn[0:srcap.shape[0], c0:c1],
                                  in_=srcap).then_inc(sems["st0"], 16)
                ndbg += 1
            for dst, srcap in ((dbg_q, q_sb), (dbg_k, k_sb),
                               (dbg_v, vaug.rearrange("p a b -> p (a b)")),
                               (dbg_avn, avn_sb.rearrange("p a b -> p (a b)")),
                               (dbg_r, r64_sb.rearrange("p a b -> p (a b)"))):
                nc.sync.dma_start(out=dst[:, :], in_=srcap).then_inc(sems["st0"], 16)
                ndbg += 1
            eng.wait_ge(sems["st0"], s.cnt["st0"] + 16 * ndbg)
            st0_extra = 16 * ndbg
        if s.emitting and finalizer:
            # Finalizer: once PE/ACT/DVE signal completion on fin, subtract
            # each semaphore's per-run total so repeat executions of the
            # loaded NEFF start from zero (device sems persist globally).
            eng.wait_ge(sems["fin"], 3)
            subs = ([("pe", totals["pe"]), ("act", totals["act"]),
                     ("dve", totals["dve"]), ("dw", 32),
                     ("st0", s.cnt["st0"] + st0_extra),
                     ("st1", s.cnt["st1"])] +
                    [(f"dxc{t}{j}", 16) for t in range(2) for j in range(4)] +
                    [("fin", 3)])
            for name, tot in subs:
                _sub_update(eng.wait_ge(sems["fin"], 3), sems[name], tot)

    def gen_pe(eng):
        def mm(out, lhsT, rhs, start, stop, ev=None, wait=None):
            if s.emitting:
                i = nc.tensor.matmul(out, lhsT, rhs, start=start, stop=stop,
                                     skip_group_check=True)
                s.attach(i, "pe", 1, ev=ev, wait=wait)
            else:
                s.bump("pe", 1, ev)

        zero_sems(eng, ["pe", "fin"])
        if s.emitting:
            eng.wait_ge(sems["dw"], 32)
        # GroupNorm cross-partition reductions. The trailing dummy matmuls
        # act as settling barriers: their systolic stream drains after the
        # real ones', so the sem inc implies the real results landed in PSUM.
        for t in range(2):
            mm(gs_ps, gm_w[t], st2_sb[:, t, :], start=(t == 0), stop=(t == 1),
               wait=f"stats2_{t}")
        mm(paux[0:1, 6:8], gm_w[0][:, 0:1], st2_sb[:, 0, :], True, True,
           ev="mm_gs")
        for t in range(2):
            mm(cb_ps[t], bm_w[t], gst2_sb, True, True,
               wait="gstat2" if t == 0 else None)
            mm(paux[0:1, 6:8], bm_w[t][:, 0:1], gst2_sb, True, True,
               ev=f"mm_cb{t}")
        # Q, K packed in one chain ([d, n] layout; psum rows 0:64 = q, 64:128 = k)
        for ch in range(NCH):
            xsl = [xn_sb[t][:, CHW * ch: CHW * (ch + 1)] for t in range(2)]
            w0 = f"xnc{ch}" if ch == 0 else f"kcopy{ch-1}"
            mm(pqk, wqk_w[0], xsl[0], True, False, wait=w0)
            mm(pqk, wqk_w[1], xsl[1], False, True, ev=f"mm_qk{ch}",
               wait=f"xnc{ch}" if ch > 0 else None)
        # V^T blocks ([m, d] layout): lhsT = xn slice (stationary), rhs = WvT.
        # Only the first VLOOK blocks are computed in the preamble; the rest
        # interleave into the attention loop (PE has slack there since the
        # loop is ACT-bound on hardware), shortening the serial preamble.
        def mm_v(mb):
            xsl = [xn_sb[t][:, MBW * mb: MBW * (mb + 1)] for t in range(2)]
            o = pv[mb % 2][:, 0:64]
            mm(o, xsl[0], wv_w[0], True, False,
               wait=f"vcopy{mb-2}" if mb >= 2 else None)
            mm(o, xsl[1], wv_w[1], False, True, ev=f"mm_v{mb}")

        for mb in range(VLOOK):
            mm_v(mb)
        # attention: mm_s leads; mm_av trails by `lag`; the proj tail trails
        # a further TAIL_DELAY iterations so the DVE normalize chain overlaps
        # the next chunk's m-loop instead of stalling the in-order PE stream.
        if s.emitting:
            eng.wait_ge(sems["dve"], s.wval("kcopy7")[1])
        # pair-granular emission: the two row-packed mm_s of a pair sit
        # back-to-back so their 64-row groups execute concurrently in the PE.
        for e2 in range(0, NE + lag + TAIL_DELAY, 2):
            j = e2 // 2
            for mb in (2 * j + VLOOK, 2 * j + 1 + VLOOK):
                if mb < NMB:
                    mm_v(mb)
            for e in (e2, e2 + 1):
                if e >= NE:
                    continue
                ch, mb = divmod(e, NMB)
                half = ps_s2[(e // 2) % 2][:, CHW * (e % 2): CHW * (e % 2 + 1)]
                rows = slice(0, 64) if e % 2 == 0 else slice(64, 128)
                mm(half, k_sb[rows, MBW * mb: MBW * (mb + 1)],
                   q_sb[rows, CHW * ch: CHW * (ch + 1)], True, True,
                   ev=f"mm_s{e}", wait=f"exp{e - 4}" if e >= 4 else None)
            for e in (e2, e2 + 1):
                ee = e - lag
                if not (0 <= ee < NE):
                    continue
                ch, mb = divmod(ee, NMB)
                if mb == 0 and ch >= 2 and s.emitting:
                    eng.wait_ge(sems["dve"], s.wval(f"avn{ch-2}")[1])
                if ch == 0 and s.emitting:
                    eng.wait_ge(sems["dve"], s.wval(f"vcopy{mb}")[1])
                mm(ps_o[ch % 2][0:65, :], vaug[:, mb, 0:65],
                   pt_sb[:, (ee // 2) % 4, CHW * (ee % 2): CHW * (ee % 2 + 1)],
                   mb == 0, mb == NMB - 1,
                   ev=f"mm_av{ee}", wait=f"exp{ee}")
            for e in (e2, e2 + 1):
                et = e - lag - TAIL_DELAY
                if et >= 0 and et % NMB == NMB - 1:
                    ch = et // NMB
                    for t in range(2):
                        mm(ps_p[t], wp_w[t], avn_sb[:, ch % 2, :], True, True,
                           ev=f"proj{ch}_{t}", wait=f"avn{ch}" if t == 0 else None)
        if s.emitting and finalizer:
            eng.wait_ge(sems["pe"], s.cnt["pe"]).then_inc(sems["fin"], 1)

    def gen_act(eng):
        def act(out, in_, func, ev=None, wait=None, **kw):
            if s.emitting:
                i = nc.scalar.activation(out, in_, func, **kw)
                s.attach(i, "act", 1, ev=ev, wait=wait)
            else:
                s.bump("act", 1, ev)

        zero_sems(eng, ["act"])
        if s.emitting:
            eng.wait_ge(sems["dw"], 32)
        # Warm-up sqrt on the (loaded) eps value: walrus inserts the sqrt
        # table-set load before the first Sqrt, so doing a throwaway one here
        # hides the ~2.7us load under the x DMA instead of paying it on the
        # GroupNorm critical chain at the real sqrt below.
        act(warm_sb, vb_sb[0:1, VB_EPS:VB_EPS + 1], AF.Sqrt,
            bias=vb_sb[0:1, VB_EPS:VB_EPS + 1])
        act(g8_sb[:, 3:4], g8_sb[:, 2:3], AF.Sqrt,
            bias=vb_sb[0:8, VB_EPS:VB_EPS + 1], ev="sqrt8", wait="var8")
        # warm-up Exp (input = eps, output discarded): hoists the exp
        # table-set load to right after the sqrt instead of delaying exp(0)
        act(warm_sb, vb_sb[0:1, VB_EPS:VB_EPS + 1], AF.Exp)
        for j in range(NE // 2):
            e0, e1 = 2 * j, 2 * j + 1
            if "smallexp" in ablate:
                act(pt_sb[0:1, j % 4, 0:1], ps_s2[j % 2][0:1, 0:1], AF.Exp,
                    ev=None, wait=f"mm_s{e1}")
            else:
                act(pt_sb[:, j % 4, :], ps_s2[j % 2], AF.Exp,
                    ev=None, wait=f"mm_s{e1}")
            if not s.emitting:
                # the pair op satisfies both halves' consumers
                s.ev[f"exp{e0}"] = ("act", s.cnt["act"])
                s.ev[f"exp{e1}"] = ("act", s.cnt["act"])
        if s.emitting and finalizer:
            eng.wait_ge(sems["act"], s.cnt["act"]).then_inc(sems["fin"], 1)

    def gen_dve(eng):
        def dve(fn, *args, ev=None, wait=None, **kw):
            # The DVE pipeline does not interlock same-engine RAW hazards
            # through SBUF for short ops; when no cross-engine wait is needed,
            # wait on our own counter (= predecessor's completion) instead.
            if s.emitting:
                i = fn(*args, **kw)
                if self_waits and wait is None and s.cnt["dve"] > 0:
                    i._wait_ge(self_sem, s.cnt["dve"])
                s.attach(i, "dve", 1, ev=ev, wait=wait)
            else:
                s.bump("dve", 1, ev)
        self_sem = sems["dve"]

        V = nc.vector
        zero_sems(eng, ["dve"])
        dve(V.memset, vaug[:, :, 64:65], 1.0)
        # GroupNorm stats
        for t in range(2):
            for i8 in range(8):
                dve(V.bn_stats, stats_sb2[t][:, i8, :],
                    x_sb[t][:, CHW * i8: CHW * (i8 + 1)],
                    wait=f"x{t}c{i8 // 2}")
            dve(V.bn_aggr, mv_sb, stats_sb2[t])
            dve(V.tensor_copy, st2_sb[:, t, 0:1], mv_sb[:, 0:1])
            dve(V.tensor_mul, musq_sb, mv_sb[:, 0:1], mv_sb[:, 0:1])
            dve(V.tensor_add, st2_sb[:, t, 1:2], musq_sb, mv_sb[:, 1:2],
                ev=f"stats2_{t}")
        # group stats -> per-group (mu, rstd)
        dve(V.tensor_scalar_mul, g8_sb[:, 0:2], gs_ps, 1.0 / 32.0, wait="mm_gs")
        dve(V.tensor_mul, g8_sb[:, 5:6], g8_sb[:, 0:1], g8_sb[:, 0:1])
        dve(V.tensor_sub, g8_sb[:, 2:3], g8_sb[:, 1:2], g8_sb[:, 5:6], ev="var8")
        dve(V.reciprocal, g8_sb[:, 4:5], g8_sb[:, 3:4], wait="sqrt8")
        dve(V.tensor_copy, gst2_sb[:, 0:1], g8_sb[:, 0:1])
        dve(V.tensor_copy, gst2_sb[:, 1:2], g8_sb[:, 4:5], ev="gstat2")
        # per-channel affine
        if s.emitting:
            eng.wait_ge(sems["dw"], 32)
        for t in range(2):
            dve(V.tensor_mul, coef_sb[:, t, 0:1], cb_ps[t][:, 1:2], gnw_v[t],
                wait=f"mm_cb{t}")
            dve(V.tensor_mul, tmp1_sb, cb_ps[t][:, 0:1], coef_sb[:, t, 0:1])
            dve(V.tensor_sub, coef_sb[:, t, 1:2], gnb_v[t], tmp1_sb)
        if s.emitting:
            eng.wait_ge(sems["dve"], s.cnt["dve"])  # coef chains settled
        if chunk_aff:
            for ch in range(NCH):
                for t in range(2):
                    cs = slice(CHW * ch, CHW * (ch + 1))
                    dve(V.tensor_scalar, xn_sb[t][:, cs], x_sb[t][:, cs],
                        coef_sb[:, t, 0:1], coef_sb[:, t, 1:2],
                        op0=OP.mult, op1=OP.add,
                        ev=f"xnc{ch}" if t == 1 else None,
                        wait=f"x{t}c{ch // 2}")
        else:
            for t in range(2):
                dve(V.tensor_scalar, xn_sb[t], x_sb[t],
                    coef_sb[:, t, 0:1], coef_sb[:, t, 1:2],
                    op0=OP.mult, op1=OP.add, ev=f"xn_t{t}")
            if not s.emitting:
                for ch in range(NCH):
                    s.ev[f"xnc{ch}"] = s.ev["xn_t1"]
        # qk copies (+bias, ->bf16), replicated into both partition halves so
        # row-packed QK matmul pairs can read their operands at base 0 and 64.
        for ch in range(NCH):
            cs = slice(CHW * ch, CHW * (ch + 1))
            dve(V.tensor_scalar_add, q_sb[0:64, cs], pqk[0:64, :], bq_v,
                wait=f"mm_qk{ch}")
            dve(V.tensor_scalar_add, q_sb[64:128, cs], pqk[0:64, :], bq_v,
                wait=f"mm_qk{ch}")
            dve(V.tensor_scalar_add, k_sb[0:64, cs], pqk[64:128, :], bk_v,
                wait=f"mm_qk{ch}")
            dve(V.tensor_scalar_add, k_sb[64:128, cs], pqk[64:128, :], bk_v,
                ev=f"kcopy{ch}", wait=f"mm_qk{ch}")
        for mb in range(NMB):
            dve(V.tensor_copy, vaug[:, mb, 0:64], pv[mb % 2][:, 0:64],
                ev=f"vcopy{mb}", wait=f"mm_v{mb}")
        s.bump("dve", 0, ev="pre_dve_end")
        # attention loop tail ops; reciprocal reads psum row 64 cross-base
        for ch in range(NCH):
            r = ch % 2
            dve(V.reciprocal, r64_sb[0:1, r, :], ps_o[r][64:65, :],
                ev=f"recip{ch}", wait=f"mm_av{32 * ch + 31}")
            dve(V.stream_shuffle, r64_sb[0:32, r, :], r64_sb[0:32, r, :],
                [0] * 32)
            dve(V.tensor_copy, r64_sb[32:64, r, :], r64_sb[0:32, r, :])
            dve(V.tensor_tensor, avn_sb[:, r, :], ps_o[r][0:64, :],
                r64_sb[:, r, :], op=OP.mult, ev=f"avn{ch}")
            if ch >= 2 and s.emitting:
                wsem, wval = s.ev[f"store{ch - 2}_1"]
                eng.wait_ge(sems[wsem], wval)
            for t in range(2):
                cs = slice(CHW * ch, CHW * (ch + 1))
                dve(V.scalar_tensor_tensor, ost_sb[:, r, t, :], x_sb[t][:, cs],
                    0.25, ps_p[t], op0=OP.mult, op1=OP.add, wait=f"proj{ch}_{t}")
                dve(V.tensor_scalar_add, ost_sb[:, r, t, :], ost_sb[:, r, t, :],
                    bp_v[t], ev=f"ocopy{ch}_{t}")
        if s.emitting and finalizer:
            eng.wait_ge(sems["dve"], s.cnt["dve"]).then_inc(sems["fin"], 1)

    # pass 0: count and record events
    s.emitting = False
    s.reset_counts()
    gen_sync(None)
    gen_pe(None)
    gen_act(None)
    gen_dve(None)
    totals = dict(s.cnt)

    # pass 1: emit
    s.emitting = True
    s.reset_counts()
    with nc.Block() as block:
        @block.sync
        def _(eng):
            gen_sync(eng)

        @block.tensor
        def _(eng):
            gen_pe(eng)

        @block.scalar
        def _(eng):
            gen_act(eng)

        @block.vector
        def _(eng):
            gen_dve(eng)

    assert s.cnt == totals, (s.cnt, totals)
    es.close()
    return nc


_NC_CACHE = None


def _get_nc():
    global _NC_CACHE
    if _NC_CACHE is None:
        _NC_CACHE = build_module()
    return _NC_CACHE


def _prep_core_inputs(x, gn_w, gn_b, qkv_w, qkv_b, proj_w, proj_b, b, h):
    bf16 = ml_dtypes.bfloat16
    x_b = np.ascontiguousarray(x[b].reshape(C, N), dtype=np.float32)

    wb = np.zeros((128, WB_COLS), dtype=bf16)
    Wq = qkv_w[h * HD:(h + 1) * HD, :] * SCALE          # [64, 256]
    Wk = qkv_w[C + h * HD: C + (h + 1) * HD, :]
    Wv = qkv_w[2 * C + h * HD: 2 * C + (h + 1) * HD, :]
    Wp = proj_w[:, h * HD:(h + 1) * HD]                  # [256, 64]
    for t in range(2):
        rs = slice(128 * t, 128 * (t + 1))
        wb[:, WB_WQK + 128 * t: WB_WQK + 128 * t + 64] = Wq.T[rs].astype(bf16)
        wb[:, WB_WQK + 128 * t + 64: WB_WQK + 128 * (t + 1)] = Wk.T[rs].astype(bf16)
        wb[:, WB_WV + 64 * t: WB_WV + 64 * (t + 1)] = Wv.T[rs].astype(bf16)
    wb[0:64, WB_WP:WB_WP + 256] = Wp.T.astype(bf16)

    vb = np.zeros((128, VB_COLS), dtype=np.float32)
    vb[0:64, VB_BQK] = qkv_b[h * HD:(h + 1) * HD] * SCALE
    vb[64:128, VB_BQK] = qkv_b[C + h * HD: C + (h + 1) * HD]
    bv = qkv_b[2 * C + h * HD: 2 * C + (h + 1) * HD]
    bp_eff = proj_b * 0.25 + Wp @ bv   # AVn = AV/l + bv; bv passes through proj
    for t in range(2):
        rs = slice(128 * t, 128 * (t + 1))
        vb[:, VB_BP + t] = bp_eff[rs]
        vb[:, VB_GNW + t] = gn_w[rs]
        vb[:, VB_GNB + t] = gn_b[rs]
        # gmask: [128, 8] one-hot of channel's group
        ch_idx = np.arange(128) + 128 * t
        gm = np.zeros((128, 8), np.float32)
        gm[np.arange(128), ch_idx // 32] = 1.0
        vb[:, VB_GM + 8 * t: VB_GM + 8 * (t + 1)] = gm
        # bmask: [8, 128] transpose of gmask
        vb[0:8, VB_BM + 128 * t: VB_BM + 128 * (t + 1)] = gm.T
    vb[0, VB_ONES:VB_ONES + 64] = 1.0
    vb[:, VB_EPS] = EPS

    return {"x": x_b, "wb": wb, "vb": vb}


def kernel(x, gn_w, gn_b, qkv_w, qkv_b, proj_w, proj_b, _trace=False):
    x = np.asarray(x, dtype=np.float32)
    gn_w = np.asarray(gn_w, dtype=np.float32)
    gn_b = np.asarray(gn_b, dtype=np.float32)
    qkv_w = np.asarray(qkv_w, dtype=np.float32)
    qkv_b = np.asarray(qkv_b, dtype=np.float32)
    proj_w = np.asarray(proj_w, dtype=np.float32)
    proj_b = np.asarray(proj_b, dtype=np.float32)

    nc = _get_nc()
    in_maps = []
    for core in range(8):
        b, h = divmod(core, HEADS)
        in_maps.append(_prep_core_inputs(x, gn_w, gn_b, qkv_w, qkv_b,
                                         proj_w, proj_b, b, h))
    res = run_bass_kernel_spmd(nc, in_maps, core_ids=list(range(8)),
                               trace=_trace)
    out = np.zeros((B, C, N), dtype=np.float32)
    for core in range(8):
        b = core // HEADS
        out[b] += res.results[core]["out"]
    if _trace:
        kernel._last_result = res
    return out.reshape(B, C, D, H, W)



# revision 11
# speedup vs baseline: 1.3805x; 1.3805x over previous
"""Trainium2 Bass kernel for nn_Attention3D (GroupNorm -> QKV -> MHA -> proj -> residual).

Sharding: one (batch, head) pair per NeuronCore (B=2 x HEADS=4 = 8 cores).
Each core computes GroupNorm(x[b]) (recomputed per core, cheap), its head's
Q/K/V, the full 4096x4096 attention for its (b,h), the head's slice of the
output projection, plus a quarter of the residual+bias. The host sums the 4
per-head partials per batch.

v2 design (cost-model driven):
- S^T = K^T Q with keys on PSUM partitions (128 keys x 512 queries per op).
- AV is FLIPPED: out[128 queries, 65] accumulating over 32 key blocks.  This
  halves the charged PE rows (the cost model bills moving-dim size only; the
  old [65,512] layout wasted half the PSUM partitions) and puts the softmax
  denominator on the partition axis, so normalization is one reciprocal plus
  one per-partition tensor_scalar instead of a cross-partition broadcast.
- exp() is split across engines: most pairs use exact Exp on the scalar
  engine; the rest use a Schraudolph fast-exp on DVE (affine to int16,
  bits reinterpreted as bf16).  Calibrated magic constant keeps the end to
  end rel-l2 error ~4e-4 (budget 2e-2); softmax self-normalization cancels
  most of the approximation bias.
- avn [128q, 64d] is transposed back for the projection with PE transpose
  ops through a bf16 PSUM tile.
- GroupNorm stats and xn run from a bf16 copy of x (halves the critical
  preamble DMA); the f32 x streams in during the loop for the residual.
- The xn affine is split ACT (ctile 0) / GPSIMD (ctile 1); q copies on ACT,
  k copies on DVE.  K bias is dropped entirely (softmax is invariant to a
  per-query constant shift).
- PSUM: preamble tensors (pqk/paux/pv0/pv1) alias loop tensors
  (ps_t/ps_av/ps_p0/ps_p1) bank-for-bank; explicit waits order the reuse.

Raw Bass (no Tile): one embedded sem-wait and one sem-update per
instruction; scheduling uses one monotone counting semaphore per engine
with a two-pass (count, then emit) scheduler.
"""

import numpy as np
import ml_dtypes

import concourse.bass as bass
import concourse.mybir as mybir
from concourse.bass_utils import run_bass_kernel_spmd

F32 = mybir.dt.float32
BF16 = mybir.dt.bfloat16
I16 = mybir.dt.int16
AF = mybir.ActivationFunctionType
OP = mybir.AluOpType

# problem constants (hardcoded per contract)
B, C, D, H, W = 2, 256, 16, 16, 16
N = D * H * W            # 4096
HEADS = 4
HD = C // HEADS          # 64
GROUPS = 8
EPS = 1e-5
SCALE = HD ** -0.5

NCH = 8                  # query chunks of 512
CHW = 512
NMB = 32                 # key blocks of 128
MBW = 128
NPAIR = 128              # pairs of key blocks (one exp tile each)
NB = 4                   # n-blocks (128 queries) per chunk

# schedule knobs
LAG = 3                  # mm_av trails mm_s by LAG pairs
DT = 2                   # transpose trails chunk's last mm_av by DT iters
DP = 2                   # proj trails transpose by DP iters
# number of DVE fast-exp pairs per chunk (rest on ACT exact exp)
N_DVE = (4, 6, 6, 6, 6, 6, 6, 6)

# Schraudolph fast-exp: bf16 bits = round(s * FA + FB) as int16
FA = 128.0 / np.log(2.0)
FB = 127.0 * 128.0 - 7.0

# wb (bf16 weight blob) column layout
WB_WQK = 0       # [128,128] per ctile: cols 0:64 = (Wq*scale).T, 64:128 = Wk.T
WB_WV = 256      # 256:320 t0, 320:384 t1
WB_WP = 384      # 384:640  rows 0:64 WpT, row 64 = bp_eff
WB_ID = 640      # 640:768 identity
WB_COLS = 768

# vb (f32 vector blob) column layout
VB_BQ = 0        # rows 0:64 = bq*scale
VB_GNW = 1       # 1,2
VB_GNB = 3       # 3,4
VB_GM = 5        # 5:13 t0, 13:21 t1   group mask [128,8]
VB_BM = 21       # 21:149 t0, 149:277 t1  bcast mask rows 0:8 [8,128]
VB_EPS = 277
VB_COLS = 278


def _wr_update(inst, sem, val):
    u = mybir.SyncUpdate(sync_type='semaphore', id=sem.num, ant_name=None,
                         update_mode='sem-wr-imm', update_value=val)
    si = inst.ins.sync_info
    if si is None:
        inst.ins.sync_info = mybir.SyncInfo(on_wait=[], on_update=[u])
    else:
        si.on_update.append(u)
    return inst


def _sub_update(inst, sem, val):
    u = mybir.SyncUpdate(sync_type='semaphore', id=sem.num, ant_name=None,
                         update_mode='sem-sub-imm', update_value=val)
    si = inst.ins.sync_info
    if si is None:
        inst.ins.sync_info = mybir.SyncInfo(on_wait=[], on_update=[u])
    else:
        si.on_update.append(u)
    return inst


class Sched:
    """Two-pass static scheduler: pass 0 counts per-engine sem positions and
    records named events; pass 1 emits instructions with embedded waits."""

    def __init__(self):
        self.ev = {}
        self.emitting = False
        self.cnt = {}
        self.sem = {}

    def reset_counts(self, keys):
        self.cnt = {k: 0 for k in keys}

    def bump(self, key, n, ev=None):
        self.cnt[key] += n
        if not self.emitting:
            if ev is not None:
                self.ev[ev] = (key, self.cnt[key])
        return self.cnt[key]

    def attach(self, inst, key, n, ev=None, wait=None):
        if self.emitting:
            if wait is not None:
                wsem, wval = self.ev[wait]
                inst._wait_ge(self.sem[wsem], wval)
            inst.then_inc(self.sem[key], n)
        self.bump(key, n, ev)

    def wval(self, evname):
        return self.ev[evname]


SEM_KEYS = ["pe", "act", "dve", "pool", "dw", "dxb", "dxf", "st0", "st1"]


def _exp_engine_table(n_dve=N_DVE):
    """exp pair j -> 'dve' or 'act'."""
    tab = []
    for ch in range(NCH):
        n = n_dve[ch]
        pos = set(int((k + 0.5) * 16 / n) for k in range(n)) if n else set()
        for p in range(16):
            tab.append('dve' if p in pos else 'act')
    return tab


def build_module(lag=LAG, dt=DT, dp=DP, n_dve=N_DVE, zeros=True,
                 finalizer=True, self_waits=True, ramp=24):
    nc = bass.Bass()
    NITER = NPAIR + lag + dt + dp + 4
    exp_eng = _exp_engine_table(n_dve)

    xb_d = nc.dram_tensor("xb", [C, N], BF16, kind="ExternalInput")
    x_d = nc.dram_tensor("x", [C, N], F32, kind="ExternalInput")
    wb_d = nc.dram_tensor("wb", [128, WB_COLS], BF16, kind="ExternalInput")
    vb_d = nc.dram_tensor("vb", [128, VB_COLS], F32, kind="ExternalInput")
    out_d = nc.dram_tensor("out", [C, N], F32, kind="ExternalOutput")

    from contextlib import ExitStack
    es = ExitStack()

    # ---- PSUM: preamble tensors (banks 0..1), freed then aliased by
    # ps_s2[0] whose first write (pair 14) postdates all preamble reads ----
    with ExitStack() as pre:
        pqk = pre.enter_context(nc.psum_tensor("pqk", [128, CHW], F32)).ap()
        paux = pre.enter_context(nc.psum_tensor("paux", [128, CHW], F32)).ap()
    gs_ps = paux[0:8, 0:2]
    cb_ps = [paux[:, 2:4], paux[:, 4:6]]

    # ---- PSUM: loop tensors (8 banks total) ----
    with ExitStack() as lp:
        ps_s2 = [lp.enter_context(nc.psum_tensor(f"ps{i}", [128, 2 * CHW], F32)).ap()
                 for i in range(3)]
        ps_av = lp.enter_context(nc.psum_tensor("pav", [128, NB, 65], F32)).ap()
        ps_p = lp.enter_context(nc.psum_tensor("pp", [128, CHW], F32)).ap()
    # V^T staging slots ([128, 4x64] f32): preamble groups 0,1 share the
    # paux corner; loop groups alternate the two halves of the proj bank
    # (all V staging completes before the first projection).
    def pv_slot(g):
        if g < 2:
            return paux[:, 256:512]
        return ps_p[:, 0:256] if g % 2 == 0 else ps_p[:, 256:512]

    # S^T pair-tile rotation: pairs 0..13 rotate tiles 1,2 (tile 0 aliases
    # the preamble pqk/paux banks and is joined once those are dead).
    def tile(j):
        return 1 + (j % 2) if j < 14 else (j - 14) % 3

    def prev_pair(j):
        if j in (0, 1, 14):
            return None
        if j < 14:
            return j - 2
        return {15: 12, 16: 13}.get(j, j - 3)

    # chunk ch's avn transpose lands in the momentarily-free S tile that
    # pair 16ch+22 will reuse (bf16 view of its first bank).
    def tr_tile(ch):
        return (16 * ch + 8) % 3

    # ---- SBUF ----
    xb_sb = [es.enter_context(nc.sbuf_tensor(f"xb{t}", [128, N], BF16)).ap()
             for t in range(2)]
    x_sb = [es.enter_context(nc.sbuf_tensor(f"x{t}", [128, N], F32)).ap()
            for t in range(2)]
    xn_sb = [es.enter_context(nc.sbuf_tensor(f"xn{t}", [128, N], BF16)).ap()
             for t in range(2)]
    q_sb = es.enter_context(nc.sbuf_tensor("q", [64, N], BF16)).ap()
    k_sb = es.enter_context(nc.sbuf_tensor("k", [64, N], BF16)).ap()
    vaug = es.enter_context(nc.sbuf_tensor("vaug", [128, NMB, 65], BF16)).ap()
    pt_sb = es.enter_context(nc.sbuf_tensor("pt", [128, 4, 2 * CHW], BF16)).ap()
    avn_sb = es.enter_context(nc.sbuf_tensor("avn", [128, 2, NB, HD], BF16)).ap()
    av_sb = es.enter_context(nc.sbuf_tensor("av", [128, NB, 65], F32)).ap()
    avnT_sb = es.enter_context(nc.sbuf_tensor("avnT", [65, 2, CHW], BF16)).ap()
    r4_sb = es.enter_context(nc.sbuf_tensor("r4", [128, 2, NB], F32)).ap()
    ost_sb = es.enter_context(nc.sbuf_tensor("ost", [128, 2, 2, CHW], F32)).ap()
    wb_sb = es.enter_context(nc.sbuf_tensor("wbs", [128, WB_COLS], BF16)).ap()
    vb_sb = es.enter_context(nc.sbuf_tensor("vbs", [128, VB_COLS], F32)).ap()
    stats_sb2 = [es.enter_context(nc.sbuf_tensor(f"stats{t}", [128, 8, 6], F32)).ap()
                 for t in range(2)]
    mv_sb = es.enter_context(nc.sbuf_tensor("mv", [128, 2], F32)).ap()
    st2_sb = es.enter_context(nc.sbuf_tensor("st2", [128, 2, 2], F32)).ap()
    musq_sb = es.enter_context(nc.sbuf_tensor("musq", [128, 1], F32)).ap()
    g8_sb = es.enter_context(nc.sbuf_tensor("g8", [8, 6], F32)).ap()
    gst2_sb = es.enter_context(nc.sbuf_tensor("gst2", [8, 2], F32)).ap()
    coef_sb = es.enter_context(nc.sbuf_tensor("coef", [128, 2, 2], F32)).ap()
    tmp1_sb = es.enter_context(nc.sbuf_tensor("tmp1", [128, 1], F32)).ap()
    warm_sb = es.enter_context(nc.sbuf_tensor("warm", [1, 1], F32)).ap()

    sems = {}
    for name in SEM_KEYS + ["fin"]:
        sems[name] = es.enter_context(nc.semaphore(f"sem_{name}"))

    s = Sched()
    s.sem = sems

    wqk_w = [wb_sb[:, WB_WQK + 128 * t: WB_WQK + 128 * (t + 1)] for t in range(2)]
    wv_w = [wb_sb[:, WB_WV + 64 * t: WB_WV + 64 * (t + 1)] for t in range(2)]
    wp_w = [wb_sb[0:65, WB_WP + 128 * t: WB_WP + 128 * (t + 1)] for t in range(2)]
    ident_w = wb_sb[:, WB_ID: WB_ID + 128]
    gm_w = [vb_sb[:, VB_GM + 8 * t: VB_GM + 8 * (t + 1)] for t in range(2)]
    bm_w = [vb_sb[0:8, VB_BM + 128 * t: VB_BM + 128 * (t + 1)] for t in range(2)]
    bq_v = vb_sb[0:64, VB_BQ: VB_BQ + 1]
    gnw_v = [vb_sb[:, VB_GNW + t: VB_GNW + t + 1] for t in range(2)]
    gnb_v = [vb_sb[:, VB_GNB + t: VB_GNB + t + 1] for t in range(2)]

    def zero_sems(eng, names):
        if s.emitting and zeros:
            for name in names:
                _wr_update(eng.wait_ge(sems[name], 0), sems[name], 0)

    def wv(eng, evname):
        """Explicit (standalone) wait on a named event."""
        if s.emitting:
            wsem, wvv = s.ev[evname]
            eng.wait_ge(sems[wsem], wvv)

    # schedule placement helpers -------------------------------------------
    def qk_at(ch):   # PE: mm_qk for chunk ch (ch>=2) at this iteration
        return 2 * ch - 4

    def xn_at(ch):   # ACT/Pool: xn affine for chunk ch (ch>=3)
        return 2 * ch - 6

    def qc_at(ch):   # ACT: q copy for chunk ch (ch>=2; 0,1 in preamble)
        return 2 * ch - 4

    def kc_at(ch):   # DVE: k copy for chunk ch (ch>=1)
        return 2 * ch - 2

    def vg_at(g):    # DVE: vaug copy group g (g>=2)
        return 2 * g - 2

    def chunk_end(ch):
        return 16 * ch + 15

    # ---------------- engine programs ----------------

    def gen_sync(eng):
        def dma(key, out, in_, ev=None, wait=None):
            if s.emitting:
                i = nc.sync.dma_start(out=out, in_=in_)
                s.attach(i, key, 16, ev=ev, wait=wait)
            else:
                s.bump(key, 16, ev)

        zero_sems(eng, ["dw", "dxb", "dxf", "st0", "st1"])
        dma("dw", wb_sb, wb_d[:, :], ev="wb")
        dma("dw", vb_sb, vb_d[:, :], ev="vb")
        for t in range(2):
            for j in range(4):
                dma("dxb", xb_sb[t][:, 1024 * j:1024 * (j + 1)],
                    xb_d[128 * t:128 * (t + 1), 1024 * j:1024 * (j + 1)],
                    ev=f"xb{t}c{j}")
        for c in range(4):
            for t in range(2):
                dma("dxf", x_sb[t][:, 1024 * c:1024 * (c + 1)],
                    x_d[128 * t:128 * (t + 1), 1024 * c:1024 * (c + 1)],
                    ev=f"xf{c}{t}")
        for ch in range(NCH):
            for t in range(2):
                dma(f"st{ch % 2}",
                    out_d[128 * t:128 * (t + 1), CHW * ch: CHW * (ch + 1)],
                    ost_sb[:, ch % 2, t, :], ev=f"store{ch}_{t}",
                    wait=f"ocopy{ch}_{t}")
        if s.emitting:
            eng.wait_ge(sems["st0"], s.cnt["st0"])
            eng.wait_ge(sems["st1"], s.cnt["st1"])
        if s.emitting and finalizer:
            eng.wait_ge(sems["fin"], 4)
            subs = ([(k, totals[k]) for k in ["pe", "act", "dve", "pool"]] +
                    [("dw", 32), ("dxb", 128), ("dxf", 128),
                     ("st0", s.cnt["st0"]), ("st1", s.cnt["st1"]),
                     ("fin", 4)])
            for name, tot in subs:
                _sub_update(eng.wait_ge(sems["fin"], 4), sems[name], tot)

    def gen_pe(eng):
        def mm(out, lhsT, rhs, start, stop, ev=None, wait=None, tr=False):
            if s.emitting:
                i = nc.tensor.matmul(out, lhsT, rhs, start=start, stop=stop,
                                     is_transpose=tr or None,
                                     skip_group_check=True)
                s.attach(i, "pe", 1, ev=ev, wait=wait)
            else:
                s.bump("pe", 1, ev)

        def mm_v(b):
            # V^T block b ([128 keys, 64 d]) into slot (b%4) of group b//4
            g = b // 4
            slot = pv_slot(g)[:, 64 * (b % 4): 64 * (b % 4) + 64]
            xsl = [xn_sb[t][:, MBW * b: MBW * (b + 1)] for t in range(2)]
            w0 = f"vcopyg{g - 2}" if (g >= 2 and b % 4 == 0) else None
            if g == 1 and b % 4 == 0:
                w0 = "vcopyg0"
            mm(slot, xsl[0], wv_w[0], True, False, wait=w0)
            mm(slot, xsl[1], wv_w[1], False, True, ev=f"mm_v{b}")

        def mm_qk(ch):
            xsl = [xn_sb[t][:, CHW * ch: CHW * (ch + 1)] for t in range(2)]
            if ch >= 1:
                wv(eng, f"qcopy{ch - 1}")
                wv(eng, f"kcopy{ch - 1}")
            mm(pqk, wqk_w[0], xsl[0], True, False, wait=f"xna{ch}")
            mm(pqk, wqk_w[1], xsl[1], False, True, ev=f"mm_qk{ch}",
               wait=f"xnp{ch}")

        zero_sems(eng, ["pe", "fin"])
        if s.emitting:
            eng.wait_ge(sems["dw"], 32)
        # PSTATE ramp: keep the PE busy during the GN-stats window with tiny
        # dummy matmuls so the 2.4GHz p-state is reached before the loop.
        if ramp:
            for t in range(2):
                for i8 in range(8):
                    for r in range(ramp):
                        mm(pqk[64:65, 384:448], wb_sb[0:1, 0:1],
                           wb_sb[0:1, 0:64], True, True,
                           wait=f"statsop{t}{i8}" if r == 0 else None)
        # GroupNorm cross-partition reductions (trailing dummies settle PSUM)
        for t in range(2):
            mm(gs_ps, gm_w[t], st2_sb[:, t, :], start=(t == 0), stop=(t == 1),
               wait=f"stats2_{t}")
        mm(paux[0:1, 6:8], gm_w[0][:, 0:1], st2_sb[:, 0, :], True, True,
           ev="mm_gs")
        for t in range(2):
            mm(cb_ps[t], bm_w[t], gst2_sb, True, True,
               wait="gstat2" if t == 0 else None)
            mm(paux[0:1, 6:8], bm_w[t][:, 0:1], gst2_sb, True, True,
               ev=f"mm_cb{t}")
        # preamble QK + V groups 0,1 (xn chunks 0,1)
        mm_qk(0)
        for b in range(4):
            mm_v(b)
        mm_qk(1)
        for b in range(4, 8):
            mm_v(b)

        # ---------------- attention loop ----------------
        for i in range(NITER):
            # mm_s pair i
            if i < NPAIR:
                ch, p = divmod(i, 16)
                m0 = 2 * p
                if ch == 0 and p % 2 == 0:
                    wv(eng, f"kcopy{p // 2}")
                if p == 0:
                    wv(eng, f"qcopy{ch}")
                if i == 14:
                    # tile 0 joins the rotation: preamble banks must be dead
                    wv(eng, f"kcopy{NCH - 1}")
                    wv(eng, f"qcopy{NCH - 1}")
                    wv(eng, "vcopyg1")
                if i >= 22 and (i - 22) % 16 == 0:
                    wv(eng, f"avnT{(i - 22) // 16}")
                pj = prev_pair(i)
                ti = ps_s2[tile(i)]
                qs = q_sb[:, CHW * ch: CHW * (ch + 1)]
                mm(ti[:, 0:CHW], k_sb[:, MBW * m0: MBW * (m0 + 1)],
                   qs, True, True, wait=f"exp{pj}" if pj is not None else None)
                mm(ti[:, CHW:2 * CHW],
                   k_sb[:, MBW * (m0 + 1): MBW * (m0 + 2)],
                   qs, True, True, ev=f"mm_s{i}")
            # mm_av pair i-lag (flipped: queries on partitions)
            jj = i - lag
            if 0 <= jj < NPAIR:
                ch, p = divmod(jj, 16)
                if p == 0 and ch >= 1:
                    wv(eng, f"avcopy{ch - 1}")
                if jj < 32 and p % 2 == 0:
                    wv(eng, f"vcopyg{p // 2}")
                for mi in range(2):
                    m = 2 * p + mi
                    for nb in range(NB):
                        lhs = pt_sb[:, jj % 4,
                                    CHW * mi + 128 * nb: CHW * mi + 128 * (nb + 1)]
                        mm(ps_av[:, nb, :], lhs, vaug[:, m, :],
                           m == 0, m == NMB - 1,
                           wait=f"exp{jj}" if (mi == 0 and nb == 0) else None,
                           ev=f"mm_av{jj}" if (mi == 1 and nb == NB - 1) else None)
            # V blocks 8.. paced 2 per iteration
            for b in (8 + 2 * i, 9 + 2 * i):
                if b < NMB:
                    mm_v(b)
            # remaining QK chunks
            for ch in range(2, NCH):
                if i == qk_at(ch):
                    mm_qk(ch)
            # transpose avn into a momentarily-free S tile
            for ch in range(NCH):
                if i == chunk_end(ch) + lag + dt:
                    jp = 16 * ch + 19
                    if jp < NPAIR:
                        wv(eng, f"exp{jp}")
                    pst = ps_s2[tr_tile(ch)][0:64, 0:256].bitcast(BF16)
                    for nb in range(NB):
                        mm(pst[:, 128 * nb: 128 * (nb + 1)],
                           avn_sb[:, ch % 2, nb, :], ident_w, True, True,
                           tr=True,
                           wait=f"avnw{ch}" if nb == 0 else None,
                           ev=f"tr{ch}" if nb == NB - 1 else None)
            # projection for finished chunk (single proj bank: t0 then t1)
            for ch in range(NCH):
                for t in range(2):
                    if i == chunk_end(ch) + lag + dt + dp + 2 * t:
                        if ch == 0 and t == 0:
                            wv(eng, f"vcopyg{NMB // 4 - 1}")
                        if ch >= 1 and t == 0:
                            wv(eng, f"ocopy{ch - 1}_1")
                        if t == 1:
                            wv(eng, f"ocopy{ch}_0")
                        mm(ps_p, wp_w[t], avnT_sb[:, ch % 2, :], True, True,
                           ev=f"proj{ch}_{t}",
                           wait=f"avnT{ch}" if t == 0 else None)
        if s.emitting and finalizer:
            eng.wait_ge(sems["pe"], s.cnt["pe"]).then_inc(sems["fin"], 1)

    def gen_act(eng):
        def act(out, in_, func, ev=None, wait=None, **kw):
            if s.emitting:
                i = nc.scalar.activation(out, in_, func, **kw)
                s.attach(i, "act", 1, ev=ev, wait=wait)
            else:
                s.bump("act", 1, ev)

        def xn_t0(ch):
            cs = slice(CHW * ch, CHW * (ch + 1))
            act(xn_sb[0][:, cs], xb_sb[0][:, cs], AF.Identity,
                scale=coef_sb[:, 0, 0:1], bias=coef_sb[:, 0, 1:2],
                ev=f"xna{ch}", wait="coef0" if ch == 0 else None)

        def qcopy(ch):
            cs = slice(CHW * ch, CHW * (ch + 1))
            act(q_sb[:, cs], pqk[0:64, :], AF.Identity, bias=bq_v,
                ev=f"qcopy{ch}", wait=f"mm_qk{ch}")

        zero_sems(eng, ["act"])
        if s.emitting:
            eng.wait_ge(sems["dw"], 32)
        # warm-up sqrt + exp: hoist both activation-table loads into the
        # DMA/stats window instead of paying them on the critical chain.
        act(warm_sb, vb_sb[0:1, VB_EPS:VB_EPS + 1], AF.Sqrt,
            bias=vb_sb[0:1, VB_EPS:VB_EPS + 1])
        act(g8_sb[:, 3:4], g8_sb[:, 2:3], AF.Sqrt,
            bias=vb_sb[0:8, VB_EPS:VB_EPS + 1], ev="sqrt8", wait="var8")
        act(warm_sb, vb_sb[0:1, VB_EPS:VB_EPS + 1], AF.Exp)
        xn_t0(0)
        xn_t0(1)
        xn_t0(2)
        qcopy(0)
        qcopy(1)
        for i in range(NITER):
            for ch in range(3, NCH):
                if i == xn_at(ch):
                    xn_t0(ch)
            for ch in range(2, NCH):
                if i == qc_at(ch):
                    qcopy(ch)
            if i < NPAIR and exp_eng[i] == 'act':
                act(pt_sb[:, i % 4, :], ps_s2[tile(i)], AF.Exp,
                    ev=f"exp{i}", wait=f"mm_s{i}")
        if s.emitting and finalizer:
            eng.wait_ge(sems["act"], s.cnt["act"]).then_inc(sems["fin"], 1)

    def gen_pool(eng):
        def xn_t1(ch):
            cs = slice(CHW * ch, CHW * (ch + 1))
            if s.emitting:
                i = nc.gpsimd.tensor_scalar(
                    xn_sb[1][:, cs], xb_sb[1][:, cs],
                    coef_sb[:, 1, 0:1], coef_sb[:, 1, 1:2],
                    op0=OP.mult, op1=OP.add)
                s.attach(i, "pool", 1, ev=f"xnp{ch}",
                         wait="coef1" if ch == 0 else None)
            else:
                s.bump("pool", 1, ev=f"xnp{ch}")

        zero_sems(eng, ["pool"])
        xn_t1(0)
        xn_t1(1)
        xn_t1(2)
        for i in range(NITER):
            for ch in range(3, NCH):
                if i == xn_at(ch):
                    xn_t1(ch)
        if s.emitting and finalizer:
            eng.wait_ge(sems["pool"], s.cnt["pool"]).then_inc(sems["fin"], 1)

    def gen_dve(eng):
        def dve(fn, *args, ev=None, wait=None, **kw):
            if s.emitting:
                i = fn(*args, **kw)
                if self_waits and wait is None and s.cnt["dve"] > 0:
                    i._wait_ge(self_sem, s.cnt["dve"])
                s.attach(i, "dve", 1, ev=ev, wait=wait)
            else:
                s.bump("dve", 1, ev)
        self_sem = sems["dve"]

        V = nc.vector
        zero_sems(eng, ["dve"])
        dve(V.memset, vaug[:, :, 64:65], 1.0)
        dve(V.memset, avnT_sb[64:65, :, :], 1.0)
        # GroupNorm stats (from bf16 x)
        for t in range(2):
            for i8 in range(8):
                dve(V.bn_stats, stats_sb2[t][:, i8, :],
                    xb_sb[t][:, CHW * i8: CHW * (i8 + 1)],
                    ev=f"statsop{t}{i8}", wait=f"xb{t}c{i8 // 2}")
            dve(V.bn_aggr, mv_sb, stats_sb2[t])
            dve(V.tensor_copy, st2_sb[:, t, 0:1], mv_sb[:, 0:1])
            dve(V.tensor_mul, musq_sb, mv_sb[:, 0:1], mv_sb[:, 0:1])
            dve(V.tensor_add, st2_sb[:, t, 1:2], musq_sb, mv_sb[:, 1:2],
                ev=f"stats2_{t}")
        # group stats -> per-group (mu, rstd)
        dve(V.tensor_scalar_mul, g8_sb[:, 0:2], gs_ps, 1.0 / 32.0, wait="mm_gs")
        dve(V.tensor_mul, g8_sb[:, 5:6], g8_sb[:, 0:1], g8_sb[:, 0:1])
        dve(V.tensor_sub, g8_sb[:, 2:3], g8_sb[:, 1:2], g8_sb[:, 5:6], ev="var8")
        dve(V.reciprocal, g8_sb[:, 4:5], g8_sb[:, 3:4], wait="sqrt8")
        dve(V.tensor_copy, gst2_sb[:, 0:1], g8_sb[:, 0:1])
        dve(V.tensor_copy, gst2_sb[:, 1:2], g8_sb[:, 4:5], ev="gstat2")
        # per-channel affine coefficients
        if s.emitting:
            eng.wait_ge(sems["dw"], 32)
        for t in range(2):
            dve(V.tensor_mul, coef_sb[:, t, 0:1], cb_ps[t][:, 1:2], gnw_v[t],
                wait=f"mm_cb{t}")
            dve(V.tensor_mul, tmp1_sb, cb_ps[t][:, 0:1], coef_sb[:, t, 0:1])
            dve(V.tensor_sub, coef_sb[:, t, 1:2], gnb_v[t], tmp1_sb,
                ev=f"coef{t}")

        def kcopy(ch):
            cs = slice(CHW * ch, CHW * (ch + 1))
            dve(V.tensor_copy, k_sb[:, cs], pqk[64:128, :],
                ev=f"kcopy{ch}", wait=f"mm_qk{ch}")

        def vcopyg(g):
            dst = vaug[:, 4 * g: 4 * (g + 1), 0:64]
            src = pv_slot(g).rearrange("p (b d) -> p b d", b=4)
            dve(V.tensor_copy, dst, src, ev=f"vcopyg{g}", wait=f"mm_v{4 * g + 3}")

        kcopy(0)
        vcopyg(0)
        vcopyg(1)
        # ---------------- loop ----------------
        for i in range(NITER):
            for ch in range(1, NCH):
                if i == kc_at(ch):
                    kcopy(ch)
            for g in range(2, NMB // 4):
                if i == vg_at(g):
                    vcopyg(g)
            if i < NPAIR and exp_eng[i] == 'dve':
                if s.emitting:
                    out = pt_sb[:, i % 4, :].bitcast(I16)
                    inst = V.tensor_scalar(out, ps_s2[tile(i)], FA, FB,
                                           op0=OP.mult, op1=OP.add)
                    s.attach(inst, "dve", 1, ev=f"exp{i}", wait=f"mm_s{i}")
                else:
                    s.bump("dve", 1, ev=f"exp{i}")
            jj = i - lag
            for ch in range(NCH):
                if jj == chunk_end(ch):
                    # drain accumulators to SBUF (frees ps_av for next chunk),
                    # then normalize from SBUF (2x DVE mode there)
                    if ch >= 2 and s.emitting:
                        wv(eng, f"tr{ch - 2}")
                    dve(V.tensor_copy, av_sb, ps_av, ev=f"avcopy{ch}",
                        wait=f"mm_av{jj}")
                    dve(V.reciprocal, r4_sb[:, ch % 2, :],
                        av_sb[:, :, 64:65].rearrange("p a b -> p (a b)"))
                    for nb in range(NB):
                        dve(V.tensor_scalar, avn_sb[:, ch % 2, nb, :],
                            av_sb[:, nb, 0:64], r4_sb[:, ch % 2, nb:nb + 1],
                            None, op0=OP.mult,
                            ev=f"avnw{ch}" if nb == NB - 1 else None)
                if i == chunk_end(ch) + lag + dt + 1:
                    if ch >= 2 and s.emitting:
                        wv(eng, f"proj{ch - 2}_1")
                    pst = ps_s2[tr_tile(ch)][0:64, 0:256].bitcast(BF16)
                    dve(V.tensor_copy, avnT_sb[0:64, ch % 2, :], pst,
                        ev=f"avnT{ch}", wait=f"tr{ch}")
                for t in range(2):
                    if i == chunk_end(ch) + lag + dt + dp + 2 * t + 1:
                        cs = slice(CHW * ch, CHW * (ch + 1))
                        if s.emitting and t == 0:
                            eng.wait_ge(sems["dxf"], 16 * (2 * (ch // 2) + 2))
                        if ch >= 2 and s.emitting:
                            wv(eng, f"store{ch - 2}_{t}")
                        dve(V.scalar_tensor_tensor, ost_sb[:, ch % 2, t, :],
                            x_sb[t][:, cs], 0.25, ps_p,
                            op0=OP.mult, op1=OP.add,
                            ev=f"ocopy{ch}_{t}", wait=f"proj{ch}_{t}")
        if s.emitting and finalizer:
            eng.wait_ge(sems["dve"], s.cnt["dve"]).then_inc(sems["fin"], 1)

    # pass 0: count and record events
    s.emitting = False
    s.reset_counts(SEM_KEYS)
    gen_sync(None)
    gen_pe(None)
    gen_act(None)
    gen_pool(None)
    gen_dve(None)
    totals = dict(s.cnt)

    # pass 1: emit
    s.emitting = True
    s.reset_counts(SEM_KEYS)
    with nc.Block() as block:
        @block.sync
        def _(eng):
            gen_sync(eng)

        @block.tensor
        def _(eng):
            gen_pe(eng)

        @block.scalar
        def _(eng):
            gen_act(eng)

        @block.gpsimd
        def _(eng):
            gen_pool(eng)

        @block.vector
        def _(eng):
            gen_dve(eng)

    assert s.cnt == totals, (s.cnt, totals)
    es.close()
    return nc


_NC_CACHE = None


def _get_nc():
    global _NC_CACHE
    if _NC_CACHE is None:
        _NC_CACHE = build_module()
    return _NC_CACHE


def _prep_core_inputs(x, gn_w, gn_b, qkv_w, qkv_b, proj_w, proj_b, b, h):
    bf16 = ml_dtypes.bfloat16
    x_b = np.ascontiguousarray(x[b].reshape(C, N), dtype=np.float32)
    xb_b = x_b.astype(bf16)

    wb = np.zeros((128, WB_COLS), dtype=bf16)
    Wq = qkv_w[h * HD:(h + 1) * HD, :] * SCALE          # [64, 256]
    Wk = qkv_w[C + h * HD: C + (h + 1) * HD, :]
    Wv = qkv_w[2 * C + h * HD: 2 * C + (h + 1) * HD, :]
    Wp = proj_w[:, h * HD:(h + 1) * HD]                  # [256, 64]
    for t in range(2):
        rs = slice(128 * t, 128 * (t + 1))
        wb[:, WB_WQK + 128 * t: WB_WQK + 128 * t + 64] = Wq.T[rs].astype(bf16)
        wb[:, WB_WQK + 128 * t + 64: WB_WQK + 128 * (t + 1)] = Wk.T[rs].astype(bf16)
        wb[:, WB_WV + 64 * t: WB_WV + 64 * (t + 1)] = Wv.T[rs].astype(bf16)
    bv = qkv_b[2 * C + h * HD: 2 * C + (h + 1) * HD]
    bp_eff = proj_b * 0.25 + Wp @ bv   # bv passes through proj (sum att = 1)
    wb[0:64, WB_WP:WB_WP + 256] = Wp.T.astype(bf16)
    wb[64, WB_WP:WB_WP + 256] = bp_eff.astype(bf16)
    wb[:, WB_ID:WB_ID + 128] = np.eye(128, dtype=bf16)

    vb = np.zeros((128, VB_COLS), dtype=np.float32)
    vb[0:64, VB_BQ] = qkv_b[h * HD:(h + 1) * HD] * SCALE
    for t in range(2):
        rs = slice(128 * t, 128 * (t + 1))
        vb[:, VB_GNW + t] = gn_w[rs]
        vb[:, VB_GNB + t] = gn_b[rs]
        ch_idx = np.arange(128) + 128 * t
        gm = np.zeros((128, 8), np.float32)
        gm[np.arange(128), ch_idx // 32] = 1.0
        vb[:, VB_GM + 8 * t: VB_GM + 8 * (t + 1)] = gm
        vb[0:8, VB_BM + 128 * t: VB_BM + 128 * (t + 1)] = gm.T
    vb[:, VB_EPS] = EPS

    return {"x": x_b, "xb": xb_b, "wb": wb, "vb": vb}


def kernel(x, gn_w, gn_b, qkv_w, qkv_b, proj_w, proj_b, _trace=False):
    x = np.asarray(x, dtype=np.float32)
    gn_w = np.asarray(gn_w, dtype=np.float32)
    gn_b = np.asarray(gn_b, dtype=np.float32)
    qkv_w = np.asarray(qkv_w, dtype=np.float32)
    qkv_b = np.asarray(qkv_b, dtype=np.float32)
    proj_w = np.asarray(proj_w, dtype=np.float32)
    proj_b = np.asarray(proj_b, dtype=np.float32)

    nc = _get_nc()
    in_maps = []
    for core in range(8):
        b, h = divmod(core, HEADS)
        in_maps.append(_prep_core_inputs(x, gn_w, gn_b, qkv_w, qkv_b,
                                         proj_w, proj_b, b, h))
    res = run_bass_kernel_spmd(nc, in_maps, core_ids=list(range(8)),
                               trace=_trace)
    out = np.zeros((B, C, N), dtype=np.float32)
    for core in range(8):
        b = core // HEADS
        out[b] += res.results[core]["out"]
    if _trace:
        kernel._last_result = res
    return out.reshape(B, C, D, H, W)
